# revision 25
# baseline (speedup 1.0000x reference)
"""Trainium2 Bass kernel for nn_BG_ALRT_62921270886438 (moe_routing).

Sharding v2: core c -> (batch b = c // 4, pair p = c % 4).  Each core computes
only its pair's two nodes per active layer; the group-wise scatter-add target
of pair p is exactly E-rows [128p, 128p+128), so the per-step x update needs
only an AllGather (groups {0-3}, {4-7}) of each core's [128, T] acc slice.
lm_head is vocab-sharded 4 ways within each batch group (same output contract
as v1).  Matmuls run in fp16 (1 cycle/row vs 4 for fp32), x state stays fp32.

Self-contained: only numpy + the concourse toolchain on sys.path.
"""
import os

import numpy as np

import concourse.bacc as bacc
import concourse.tile as tile
from concourse import mybir
from concourse.alu_op_type import AluOpType
from concourse.bass_utils import run_bass_kernel_spmd

AF = mybir.ActivationFunctionType
F32 = mybir.dt.float32
F16 = mybir.dt.float16

B, T, E, G, GD, L, N, V = 2, 256, 512, 8, 64, 8, 64, 50257
HD = GD // 2          # 32, rope half
NC = 8                # cores
VSH = 4               # vocab shards per batch group
VW = (V + VSH - 1) // VSH          # 12565 raw shard width
VQ = ((VW + 511) // 512) * 512     # 12800 padded shard width
EPS = float(np.finfo(np.float32).eps)
KT = E // 128         # 4 contraction tiles over E
NVT = VQ // 512       # 25 vocab tiles of 512
NTT = T // 128        # 2 token tiles

_PROGRAM_CACHE = {}


def _tune_act_tables(arch):
    """Steer the act-table-load pass to one set for the whole step loop.

    All step-loop activations (square/ln/exp/relu/copy/identity) exist in
    `natural_log_exp_and_others`; tanh (lm head only) in `exp_and_others`.
    The pass picks the first listed set containing the function, which makes
    Ln and Exp resolve to different sets and thrash the single active table
    (~1.3us per reload).  get_activation_tables is functools.cache'd and
    returns the same dict object, so removing the overlapping functions from
    every other set (pure removals - set ids stay aligned with act_info.json)
    makes the combined set the unique choice.
    """
    from concourse.hw_specs import get_activation_tables
    tabs = get_activation_tables(arch)
    combined = tabs.get("natural_log_exp_and_others")
    if not combined:
        return
    for name, fns in tabs.items():
        if name != "natural_log_exp_and_others":
            fns.difference_update(combined)


def _build_program(active_sets):
    """active_sets: tuple of tuples - active layer list per step."""
    nc = bacc.Bacc("TRN2", target_bir_lowering=False, debug=False, num_devices=NC)
    _tune_act_tables(nc.m.arch)
    n_ls = max(sum(len(a) for a in active_sets), 1)
    groups = [[0, 1, 2, 3], [4, 5, 6, 7]]

    d_x0t = nc.dram_tensor("x0t", [E, T], F32, kind="ExternalInput")
    d_adw = nc.dram_tensor("adw", [L, 128, 512], F16, kind="ExternalInput")
    d_qkw = nc.dram_tensor("qkw", [L, 128, 256], F16, kind="ExternalInput")
    d_qpw = nc.dram_tensor("qpw", [L, 128, 256], F16, kind="ExternalInput")
    d_vww = nc.dram_tensor("vww", [L, 128, 128], F16, kind="ExternalInput")
    d_fcw = nc.dram_tensor("fcw", [L, 128, 512], F16, kind="ExternalInput")
    d_c16 = nc.dram_tensor("c16", [128, 705], F16, kind="ExternalInput")
    d_cf = nc.dram_tensor("cstf", [128, 1155], F32, kind="ExternalInput")
    d_wap = nc.dram_tensor("wapP", [128, L], F32, kind="ExternalInput")
    d_waw = nc.dram_tensor("wawP", [128, n_ls], F32, kind="ExternalInput")
    d_wmw = nc.dram_tensor("wmwP", [128, n_ls], F32, kind="ExternalInput")
    d_rw = nc.dram_tensor("rwP", [128, KT], F16, kind="ExternalInput")
    d_rb = nc.dram_tensor("rbias2", [1, 1], F32, kind="ExternalInput")
    d_lm = nc.dram_tensor("lmt", [E, VQ], F16, kind="ExternalInput")
    d_out = nc.dram_tensor("out_lg", [T, VQ], F16, kind="ExternalOutput")

    with tile.TileContext(nc) as tc:
        with tc.tile_pool(name="cst", bufs=1) as cst, \
             tc.tile_pool(name="st", bufs=1) as st, \
             tc.tile_pool(name="wk16", bufs=3) as wk16, \
             tc.tile_pool(name="wkf", bufs=2) as wkf, \
             tc.tile_pool(name="vsb", bufs=4) as vsb, \
             tc.tile_pool(name="ps", bufs=1, space="PSUM") as ps, \
             tc.tile_pool(name="dram", bufs=20, space="DRAM") as dram:

            # ---------------- constants ----------------
            c16 = cst.tile([128, 705], F16, tag="c16", name="c16")
            nc.sync.dma_start(c16[:], d_c16.ap())
            oblk = c16[:, 0:128]            # block-diag(64) of 1/64
            ocol = c16[:, 128:192]          # (128,64) ones
            oc1 = c16[:, 192:193]           # (128,1) ones
            sel2 = c16[0:2, 193:321]        # row0 -> parts 0:64, row1 -> 64:128
            onesrow = c16[0:1, 321:449]     # (1,128) ones
            tri2 = c16[:, 449:705]          # [tri | tri] fp16

            cf = cst.tile([128, 1155], F32, tag="cf", name="cf")
            nc.sync.dma_start(cf[:], d_cf.ap())
            CC2 = cf[:, 0:512]              # [C | C]
            SS2 = cf[:, 512:1024]           # [S | S]
            eps128 = cf[:, 1024:1025]
            eps1 = cf[0:1, 1024:1025]
            one_f = cf[0:1, 1025:1026]      # 1.0 (transpose identity)
            mln15 = cf[0:1, 1026:1027]      # -ln(15)
            orowf = cf[0:1, 1027:1155]      # (1,128) ones f32

            wap = cst.tile([128, L], F32, tag="wap", name="wap")
            nc.sync.dma_start(wap[:], d_wap.ap())
            waw = cst.tile([128, n_ls], F32, tag="waw", name="waw")
            nc.sync.dma_start(waw[:], d_waw.ap())
            wmw = cst.tile([128, n_ls], F32, tag="wmw", name="wmw")
            nc.sync.dma_start(wmw[:], d_wmw.ap())
            rw = cst.tile([128, KT], F16, tag="rw", name="rw")
            nc.sync.dma_start(rw[:], d_rw.ap())
            rbias2 = cst.tile([1, 1], F32, tag="rbias2", name="rbias2")
            nc.sync.dma_start(rbias2[:], d_rb.ap())

            adw, qkw, qpw, vww, fcw = [], [], [], [], []
            for l in range(L):
                a_t = cst.tile([128, 512], F16, tag=f"adw{l}", name=f"adw{l}")
                nc.sync.dma_start(a_t[:], d_adw.ap()[l])
                adw.append(a_t)
                q_t = cst.tile([128, 256], F16, tag=f"qkw{l}", name=f"qkw{l}")
                nc.sync.dma_start(q_t[:], d_qkw.ap()[l])
                qkw.append(q_t)
                p_t = cst.tile([128, 256], F16, tag=f"qpw{l}", name=f"qpw{l}")
                nc.sync.dma_start(p_t[:], d_qpw.ap()[l])
                qpw.append(p_t)
                v_t = cst.tile([128, 128], F16, tag=f"vww{l}", name=f"vww{l}")
                nc.sync.dma_start(v_t[:], d_vww.ap()[l])
                vww.append(v_t)
                f_t = cst.tile([128, 512], F16, tag=f"fcw{l}", name=f"fcw{l}")
                nc.sync.dma_start(f_t[:], d_fcw.ap()[l])
                fcw.append(f_t)

            # lm_head weights: full shard resident in SBUF, chunked DMA so the
            # prefetch never head-of-line blocks the per-step bounce DMAs.
            lmsb = []
            LCH = 1600
            for k in range(KT):
                t_ = cst.tile([128, VQ], F16, tag=f"lm{k}", name=f"lm{k}")
                lmsb.append(t_)
                for c0 in range(0, VQ, LCH):
                    nc.sync.dma_start(
                        t_[:, c0:c0 + LCH],
                        d_lm.ap()[k * 128:(k + 1) * 128, c0:c0 + LCH])

            # ---------------- state ----------------
            xT = [st.tile([128, T], F32, tag=f"xT{k}", name=f"xT{k}") for k in range(KT)]
            xr = [st.tile([128, T], F16, tag=f"xr{k}", name=f"xr{k}") for k in range(KT)]
            acc = st.tile([128, T], F32, tag="acc", name="acc")
            xg = st.tile([128, KT * T], F16, tag="xg", name="xg")
            pcont = st.tile([1, T], F32, tag="pcont", name="pcont")
            nc.vector.memset(pcont[:], 1.0)
            nc.gpsimd.memset(acc[:], 0.0)

            # initial x (rms applied host-side)
            for k in range(KT):
                nc.sync.dma_start(xT[k][:], d_x0t.ap()[k * 128:(k + 1) * 128, :])
                with nc.allow_low_precision(reason="fp16 compute"):
                    nc.vector.tensor_copy(xr[k][:], xT[k][:])

            # (no dummy warm-up AG: collectives run in order on the CC path,
            # so a dummy only delays the real step-0 gather, which absorbs the
            # cross-core launch skew just as well on its own)
            NO_CC = bool(int(os.environ.get("BASS_V2_NO_CC", "0")))

            ls_idx = 0
            with nc.allow_low_precision(reason="fp16 compute"):
                def make_unit(l, ls_i):
                    """Four emission phases for one (layer, pair) unit; the
                    step loop staggers phases across the step's units so each
                    engine queue interleaves independent work."""
                    S = {}

                    def p1():
                        H1 = ps.tile([128, 2 * T], F32, tag="H1", bufs=1, name="ps")
                        S["p_s1"] = H1[:, T:2 * T]
                        p_xi = H1[:, 0:T]
                        for k in range(KT):
                            nc.tensor.matmul(
                                p_xi[:], adw[l][:, k * 128:(k + 1) * 128],
                                xr[k][:], start=(k == 0), stop=(k == KT - 1))
                        xi = wk16.tile([128, T], F16, tag="xi", name="xi")
                        nc.vector.tensor_copy(xi[:], p_xi[:])
                        S["xi"] = xi
                        p_v = ps.tile([128, T], F32, tag="S1", bufs=1, name="ps")
                        v_sb = [None, None]
                        for s in range(2):
                            nc.tensor.matmul(
                                p_v[:, s * 128:(s + 1) * 128],
                                xi[:, s * 128:(s + 1) * 128],
                                vww[l][:], start=True, stop=True)
                            vt = vsb.tile([128, 130], F16, tag="vt", name="vt")
                            if s == 0:
                                nc.scalar.copy(vt[:, 0:64], p_v[:, 0:64])
                                nc.scalar.copy(vt[:, 65:129], p_v[:, 64:128])
                            else:
                                nc.vector.tensor_copy(vt[:, 0:64], p_v[:, 128:192])
                                nc.vector.tensor_copy(vt[:, 65:129], p_v[:, 192:256])
                            nc.gpsimd.memset(vt[:, 64:65], 1.0)
                            nc.gpsimd.memset(vt[:, 129:130], 1.0)
                            v_sb[s] = vt
                        S["v_sb"] = v_sb
                        p_qk = ps.tile([128, 2 * T], F32, tag="A", bufs=2, name="ps")
                        p_qp = ps.tile([128, 2 * T], F32, tag="A", bufs=2, name="ps")
                        for o in range(2):
                            nc.tensor.matmul(p_qk[:, o * T:(o + 1) * T],
                                             qkw[l][:, o * 128:(o + 1) * 128],
                                             xi[:], start=True, stop=True)
                            nc.tensor.matmul(p_qp[:, o * T:(o + 1) * T],
                                             qpw[l][:, o * 128:(o + 1) * 128],
                                             xi[:], start=True, stop=True)
                        S["p_qk"], S["p_qp"] = p_qk, p_qp

                    def p2():
                        p_qk, p_qp = S["p_qk"], S["p_qp"]
                        sq = wk16.tile([128, 2 * T], F16, tag="sq", name="sq")
                        nc.scalar.activation(sq[:], p_qk[:], AF.Square)
                        p_ms = ps.tile([128, 2 * T], F32, tag="A", bufs=2, name="ps")
                        nc.tensor.matmul(p_ms[:], oblk, sq[:], start=True, stop=True)
                        lnm = wkf.tile([128, 2 * T], F32, tag="srt", name="lnm")
                        nc.scalar.activation(lnm[:], p_ms[:], AF.Ln, bias=eps128)
                        rsq = wk16.tile([128, 2 * T], F16, tag="rsq", name="rsq")
                        nc.scalar.activation(rsq[:], lnm[:], AF.Exp, scale=-0.5)
                        t1 = wk16.tile([128, 2 * T], F16, bufs=2, tag="t1", name="t1")
                        nc.vector.tensor_tensor(t1[:], p_qk[:], CC2, AluOpType.mult)
                        t2 = wk16.tile([128, 2 * T], F16, bufs=2, tag="t2", name="t2")
                        nc.vector.tensor_tensor(t2[:], p_qp[:], SS2, AluOpType.mult)
                        rop = wk16.tile([128, 2 * T], F16, bufs=2, tag="rop", name="rop")
                        nc.vector.tensor_tensor(rop[:], t1[:], t2[:], AluOpType.add)
                        qt = wk16.tile([128, T], F16, tag="qt", name="qt")
                        kt = wk16.tile([128, 2 * T], F16, tag="kt", name="kt")
                        nc.gpsimd.memset(kt[64:128, 0:T], 0.0)
                        nc.gpsimd.memset(kt[0:64, T:2 * T], 0.0)
                        for o in range(2):
                            orows = slice(64 * o, 64 * o + 64)
                            nc.vector.tensor_tensor(
                                qt[orows, :], rop[0:64, o * T:(o + 1) * T],
                                rsq[0:64, o * T:(o + 1) * T], AluOpType.mult)
                            nc.vector.tensor_tensor(
                                kt[orows, o * T:(o + 1) * T],
                                rop[64:128, o * T:(o + 1) * T],
                                rsq[64:128, o * T:(o + 1) * T], AluOpType.mult)
                        p_s0 = ps.tile([128, 2 * T], F32, tag="A", bufs=2, name="ps")
                        p_s1 = S["p_s1"]
                        for o in range(2):
                            nc.tensor.matmul(p_s0[:, o * T:(o + 1) * T],
                                             kt[:, o * T:o * T + 128], qt[:],
                                             start=True, stop=True)
                            nc.tensor.matmul(p_s1[:, o * 128:(o + 1) * 128],
                                             kt[:, o * T + 128:(o + 1) * T],
                                             qt[:, 128:256],
                                             start=True, stop=True)
                        em0 = wk16.tile([128, 2 * T], F16, tag="em0", name="em0")
                        nc.scalar.activation(em0[:], p_s0[:], AF.Exp, scale=0.125)
                        em1 = wk16.tile([128, T], F16, tag="em1", name="em1")
                        nc.scalar.activation(em1[:], p_s1[:], AF.Exp, scale=0.125)
                        m0 = wk16.tile([128, T], F16, tag="m0", name="m0")
                        nc.gpsimd.tensor_tensor(m0[:, 0:128], em0[:, 0:128],
                                                tri2[:, 0:128], AluOpType.mult)
                        nc.gpsimd.tensor_tensor(m0[:, 128:256], em0[:, T:T + 128],
                                                tri2[:, 0:128], AluOpType.mult)
                        m1 = wk16.tile([128, T], F16, tag="m1", name="m1")
                        nc.gpsimd.tensor_tensor(m1[:], em1[:], tri2, AluOpType.mult)
                        S["em0"], S["m0"], S["m1"] = em0, m0, m1

                    def p3():
                        em0, m0, m1 = S["em0"], S["m0"], S["m1"]
                        v_sb, xi = S["v_sb"], S["xi"]
                        S2 = ps.tile([128, 2 * T], F32, tag="S2", bufs=1, name="ps")
                        p_att = [S2[0:65, 0:T], S2[0:65, T:2 * T]]
                        for o in range(2):
                            pa = p_att[o]
                            nc.tensor.matmul(pa[:, 0:128],
                                             v_sb[0][:, o * 65:(o + 1) * 65],
                                             m0[:, o * 128:(o + 1) * 128],
                                             start=True, stop=True)
                            nc.tensor.matmul(pa[:, 128:256],
                                             v_sb[0][:, o * 65:(o + 1) * 65],
                                             em0[:, o * T + 128:(o + 1) * T],
                                             start=True, stop=False)
                            nc.tensor.matmul(pa[:, 128:256],
                                             v_sb[1][:, o * 65:(o + 1) * 65],
                                             m1[:, o * 128:(o + 1) * 128],
                                             start=False, stop=True)
                        rcl = wkf.tile([1, 2 * T], F32, bufs=1, tag="rcl", name="rcl")
                        nc.scalar.activation(rcl[:], S2[64:65, 0:2 * T], AF.Ln)
                        rc2 = wkf.tile([1, 2 * T], F32, bufs=1, tag="rc2", name="rc2")
                        nc.scalar.activation(rc2[:], rcl[:], AF.Exp, scale=-1.0)
                        H2 = ps.tile([128, 2 * T], F32, tag="H2", bufs=1, name="ps")
                        nc.tensor.matmul(H2[:], orowf, rc2[:], start=True, stop=True)
                        att_sb = wk16.tile([128, T], F16, tag="att", name="att")
                        nc.scalar.copy(att_sb[0:64, :], p_att[0][0:64, :])
                        nc.scalar.copy(att_sb[64:128, :], p_att[1][0:64, :])
                        tt = wk16.tile([128, T], F16, tag="tt", name="tt")
                        nc.vector.tensor_tensor(tt[0:64, :], att_sb[0:64, :],
                                                H2[0:64, 0:T], AluOpType.mult)
                        nc.vector.tensor_tensor(tt[64:128, :], att_sb[64:128, :],
                                                H2[64:128, T:2 * T], AluOpType.mult)
                        xim = wk16.tile([128, T], F16, tag="xim", name="xim")
                        nc.vector.scalar_tensor_tensor(
                            xim[:], tt[:], wap[:, l:l + 1], xi[:],
                            AluOpType.mult, AluOpType.add)
                        nc.vector.scalar_tensor_tensor(
                            acc[:], tt[:], waw[:, ls_i:ls_i + 1], acc[:],
                            AluOpType.mult, AluOpType.add)
                        S["xim"] = xim

                    def p4():
                        xim = S["xim"]
                        sqm = wk16.tile([128, T], F16, tag="sqm", name="sqm")
                        nc.gpsimd.tensor_tensor(sqm[:], xim[:], xim[:],
                                                AluOpType.mult)
                        p_mq = ps.tile([128, T], F32, tag="H3", bufs=1, name="ps")
                        nc.tensor.matmul(p_mq[:], oblk, sqm[:], start=True, stop=True)
                        lnm2 = wkf.tile([128, T], F32, bufs=1, tag="pre", name="lnm2")
                        nc.scalar.activation(lnm2[:], p_mq[:], AF.Ln, bias=eps128)
                        rec2 = wk16.tile([128, T], F16, tag="rec2", name="rec2")
                        nc.scalar.activation(rec2[:], lnm2[:], AF.Exp, scale=-1.0)
                        p_sr01 = ps.tile([64, 2 * T], F32, tag="H3", bufs=1, name="ps")
                        p_srs = [p_sr01[:, 0:T], p_sr01[:, T:2 * T]]
                        for o in range(2):
                            p_fc = ps.tile([128, 2 * T], F32, tag="B", bufs=1, name="ps")
                            for h in range(2):
                                nc.tensor.matmul(
                                    p_fc[:, h * T:(h + 1) * T],
                                    fcw[l][:, o * 256 + h * 128:o * 256 + (h + 1) * 128],
                                    xim[:], start=True, stop=True)
                            frel = wk16.tile([128, 2 * T], F16, tag="frel", name="frel")
                            nc.scalar.activation(frel[:], p_fc[:], AF.Relu)
                            rsq2 = wk16.tile([128, 2 * T], F16, tag="rsq2", name="rsq2")
                            nc.gpsimd.tensor_tensor(rsq2[:], frel[:], frel[:],
                                                    AluOpType.mult)
                            nc.tensor.matmul(p_srs[o][:], ocol, rsq2[:, 0:T],
                                             start=True, stop=False)
                            nc.tensor.matmul(p_srs[o][:], ocol, rsq2[:, T:2 * T],
                                             start=False, stop=True)
                        hm = wk16.tile([128, T], F16, tag="hm", name="hm")
                        nc.vector.tensor_tensor(hm[0:64, :], p_srs[0][:],
                                                rec2[0:64, :], AluOpType.mult)
                        nc.vector.tensor_tensor(hm[64:128, :], p_srs[1][:],
                                                rec2[64:128, :], AluOpType.mult)
                        nc.vector.scalar_tensor_tensor(
                            acc[:], hm[:], wmw[:, ls_i:ls_i + 1], acc[:],
                            AluOpType.mult, AluOpType.add)

                    return [p1, p2, p3, p4]

                for t, layers in enumerate(active_sets):
                    units = [make_unit(l, ls_idx + j) for j, l in enumerate(layers)]
                    ls_idx += len(layers)
                    NPH = 4
                    for k in range(len(units) + NPH - 1):
                        for j in range(len(units)):
                            phn = k - j
                            if 0 <= phn < NPH:
                                units[j][phn]()

                    # ---- step sync: scale acc by pcont, AllGather, update x ----
                    p_pc = ps.tile([128, T], F32, tag="H1", bufs=1, name="ps")
                    nc.tensor.matmul(p_pc[:], orowf, pcont[:], start=True, stop=True)
                    acc2 = wk16.tile([128, T], F16, bufs=1, tag="acc2", name="acc2")
                    nc.vector.tensor_tensor(acc2[:], acc[:], p_pc[:], AluOpType.mult)
                    nc.gpsimd.memset(acc[:], 0.0)
                    b_in = dram.tile([128, T], F16, tag="bin", name=f"bin{t}")
                    b_out = dram.tile([KT * 128, T], F16, tag="bout", name=f"bout{t}")
                    nc.sync.dma_start(b_in[:], acc2[:])
                    if not NO_CC:
                        nc.gpsimd.collective_compute(
                            "AllGather", mybir.AluOpType.bypass, replica_groups=groups,
                            ins=[b_in[:].opt()], outs=[b_out[:].opt()])
                        for k in range(KT):
                            nc.sync.dma_start(xg[:, k * T:(k + 1) * T],
                                              b_out[k * 128:(k + 1) * 128, :])
                    else:
                        for k in range(KT):
                            nc.sync.dma_start(xg[:, k * T:(k + 1) * T], b_in[:])
                    for k in range(KT):
                        nc.vector.tensor_tensor(xT[k][:], xT[k][:],
                                                xg[:, k * T:(k + 1) * T],
                                                AluOpType.add)
                        nc.vector.tensor_copy(xr[k][:], xT[k][:])

                    # ---- router: pcont *= 1 - sigmoid(x@rw + rb) ----
                    p_ph = ps.tile([1, T], F32, tag="H3", bufs=1, name="ps")
                    for k in range(KT):
                        nc.tensor.matmul(p_ph[:], rw[:, k:k + 1], xr[k][:],
                                         start=(k == 0), stop=(k == KT - 1))
                    ez = wkf.tile([1, T], F32, bufs=1, tag="th", name="ez")
                    nc.scalar.activation(ez[:], p_ph[:], AF.Exp, bias=rbias2[:])
                    ez1 = wkf.tile([1, T], F32, bufs=1, tag="omp", name="ez1")
                    nc.vector.tensor_scalar(ez1[:], ez[:], 1.0, 1.0,
                                            AluOpType.mult, AluOpType.add)
                    lz = wkf.tile([1, T], F32, bufs=1, tag="lz", name="lz")
                    nc.scalar.activation(lz[:], ez1[:], AF.Ln)
                    omp = wkf.tile([1, T], F32, bufs=1, tag="omp2", name="omp")
                    nc.scalar.activation(omp[:], lz[:], AF.Exp, scale=-1.0)
                    nc.vector.tensor_tensor(pcont[:], pcont[:], omp[:],
                                            AluOpType.mult)

                # ---------------- final rms + lm_head ----------------
                p_mr = ps.tile([1, T], F32, tag="H3", bufs=1, name="ps")
                for k in range(KT):
                    sqf = wk16.tile([128, T], F16, tag="sqf", name="sqf")
                    nc.scalar.activation(sqf[:], xr[k][:], AF.Square)
                    nc.tensor.matmul(p_mr[:], oc1, sqf[:],
                                     start=(k == 0), stop=(k == KT - 1))
                lnf = wkf.tile([1, T], F32, bufs=1, tag="rr", name="lnf")
                nc.scalar.activation(lnf[:], p_mr[:], AF.Ln, bias=eps1,
                                     scale=1.0 / E)
                rr15 = wkf.tile([1, T], F32, bufs=1, tag="rr15", name="rr15")
                nc.scalar.activation(rr15[:], lnf[:], AF.Exp, scale=-0.5,
                                     bias=mln15)
                rcol = []
                for i in range(NTT):
                    p_tr = ps.tile([128, 1], F32, tag="S1", bufs=1, name="ptr")
                    nc.tensor.transpose(p_tr[:], rr15[:, i * 128:(i + 1) * 128],
                                        one_f)
                    rc = st.tile([128, 1], F32, tag=f"rcol{i}", name=f"rcol{i}")
                    nc.scalar.copy(rc[:], p_tr[:])
                    rcol.append(rc)

                for i in range(NTT):
                    for v in range(NVT):
                        p_lg = ps.tile([128, 512], F32, tag="A", bufs=2, name="ps")
                        for k in range(KT):
                            nc.tensor.matmul(
                                p_lg[:], xr[k][:, i * 128:(i + 1) * 128],
                                lmsb[k][:, v * 512:(v + 1) * 512],
                                start=(k == 0), stop=(k == KT - 1))
                        lth = wk16.tile([128, 512], F16, tag="lth", name="lth")
                        nc.scalar.activation(lth[:], p_lg[:], AF.Tanh,
                                             scale=rcol[i][:])
                        nc.sync.dma_start(
                            d_out.ap()[i * 128:(i + 1) * 128,
                                       v * 512:(v + 1) * 512],
                            lth[:])

    nc.compile()
    return nc


def _rms_np(x):
    return x * (1.0 / np.sqrt(np.mean(x * x, axis=-1, keepdims=True) + EPS))


def _host_prep(idx, n_steps, wte, adapters, qkv_w, attn_proj, mlp_fc, mlp_proj,
               dep, router_w, router_b, lm_head_w):
    idx = np.asarray(idx)
    wte = np.asarray(wte, np.float32)
    adapters = np.asarray(adapters, np.float32)
    qkv_w = np.asarray(qkv_w, np.float32)
    attn_proj = np.asarray(attn_proj, np.float32)
    mlp_fc = np.asarray(mlp_fc, np.float32)
    mlp_proj = np.asarray(mlp_proj, np.float32)
    dep = np.asarray(dep, np.float32)
    router_w = np.asarray(router_w, np.float32).reshape(E, 1)
    router_b = np.asarray(router_b, np.float32).reshape(-1)
    lm_head_w = np.asarray(lm_head_w, np.float32)
    ns = int(n_steps)

    dp = np.maximum(dep, 0.0)
    depths = np.zeros((N,), np.float32)
    for _ in range(L):
        depths = (dp @ (depths + 1.0)).astype(np.float32)

    w_eff = np.zeros((ns, N), np.float32)
    active_sets = []
    for t in range(ns):
        td = t * (L / ns)
        w_all = np.exp(-np.abs(depths - np.float32(td))).astype(np.float32)
        w = np.where(w_all > 0.15, w_all, 0.0).astype(np.float32)
        w_eff[t] = w
        active_sets.append(tuple(sorted({n // G for n in range(N) if w[n] > 0})))
    active_sets = tuple(active_sets)
    n_ls = max(sum(len(a) for a in active_sets), 1)

    # fold the group-slice identity into the adapters
    adapters_f = adapters.copy()
    for n in range(N):
        g = n % G
        adapters_f[n, :, g * GD:(g + 1) * GD] += np.eye(GD, dtype=np.float32)

    # rope permutation of the q/k OUTPUT index: out j <- out (j+32)%64 within
    # each 64-block (q block and k block separately)
    perm64 = (np.arange(GD) + HD) % GD
    perm128 = np.concatenate([perm64, GD + perm64])

    w_ap = attn_proj.sum(axis=2)
    w_mp = mlp_proj.sum(axis=2)

    # per-pair weight payloads
    payload = []
    for p in range(VSH):
        adw = np.zeros((L, 128, 512), np.float16)
        qkwA = np.zeros((L, 128, 256), np.float16)
        qpwA = np.zeros((L, 128, 256), np.float16)
        vwwA = np.zeros((L, 128, 128), np.float16)
        fcwA = np.zeros((L, 128, 512), np.float16)
        wapP = np.zeros((128, L), np.float32)
        wawP = np.zeros((128, n_ls), np.float32)
        wmwP = np.zeros((128, n_ls), np.float32)
        for l in range(L):
            for o in range(2):
                n = l * G + 2 * p + o
                rows = slice(o * 64, (o + 1) * 64)
                for k in range(KT):
                    adw[l, :, k * 128 + o * 64: k * 128 + (o + 1) * 64] = \
                        adapters_f[n, :, k * 128:(k + 1) * 128].T
                # zero-padded full-128-contraction stationaries (node o's
                # weights live on its own 64 rows; the rest stay zero)
                qkwA[l, rows, o * 128:(o + 1) * 128] = qkv_w[n, 0:128, :].T
                qpwA[l, rows, o * 128:(o + 1) * 128] = qkv_w[n, 0:128, :].T[:, perm128]
                vwwA[l, rows, o * 64:(o + 1) * 64] = qkv_w[n, 128:192, :].T
                fcwA[l, rows, o * 256:(o + 1) * 256] = mlp_fc[n].T
                wapP[o * 64:(o + 1) * 64, l] = w_ap[n]
        ls = 0
        for tt, layers in enumerate(active_sets):
            for l in layers:
                for o in range(2):
                    n = l * G + 2 * p + o
                    wawP[o * 64:(o + 1) * 64, ls] = w_ap[n] * w_eff[tt, n]
                    wmwP[o * 64:(o + 1) * 64, ls] = w_mp[n] * w_eff[tt, n]
                ls += 1
        payload.append((adw, qkwA, qpwA, vwwA, fcwA, wapP, wawP, wmwP))

    # constants
    c16 = np.zeros((128, 705), np.float16)
    ob = np.zeros((128, 128), np.float32)
    ob[0:64, 0:64] = 1.0 / GD
    ob[64:128, 64:128] = 1.0 / GD
    c16[:, 0:128] = ob.astype(np.float16)
    c16[:, 128:192] = 1.0
    c16[:, 192:193] = 1.0
    c16[0, 193:257] = 1.0
    c16[1, 257:321] = 1.0
    c16[0, 321:449] = 1.0
    s_i = np.arange(128)[:, None]
    t_i = np.arange(128)[None, :]
    tri = (s_i <= t_i).astype(np.float16)
    c16[:, 449:577] = tri
    c16[:, 577:705] = tri

    inv_freq = 1.0 / (10000.0 ** (np.arange(0, GD, 2, dtype=np.float64) / GD))
    freqs = np.outer(np.arange(T), inv_freq)
    cosT = np.cos(freqs).astype(np.float32).T
    sinT = np.sin(freqs).astype(np.float32).T
    cstf = np.zeros((128, 1155), np.float32)
    for blk in range(4):
        cstf[blk * 32:(blk + 1) * 32, 0:256] = cosT
        cstf[blk * 32:(blk + 1) * 32, 256:512] = cosT
        cstf[blk * 32:(blk + 1) * 32, 512:768] = sinT * (1.0 if blk % 2 == 0 else -1.0)
        cstf[blk * 32:(blk + 1) * 32, 768:1024] = sinT * (1.0 if blk % 2 == 0 else -1.0)
    cstf[:, 1024] = EPS
    cstf[0, 1025] = 1.0
    cstf[0, 1026] = -np.log(15.0)
    cstf[0, 1027:1155] = 1.0

    rwP = np.zeros((128, KT), np.float16)
    for k in range(KT):
        rwP[:, k] = router_w[k * 128:(k + 1) * 128, 0].astype(np.float16)
    rbias2 = np.full((1, 1), np.float32(router_b[0]), np.float32)

    x0 = _rms_np(wte[idx])  # (B, T, E) f32

    in_maps = []
    for c in range(NC):
        b, p = c // VSH, c % VSH
        lo = p * VW
        hi = min(lo + VW, V)
        lmt = np.zeros((E, VQ), np.float16)
        lmt[:, 0:hi - lo] = lm_head_w[lo:hi, :].T.astype(np.float16)
        adw, qkwA, qpwA, vwwA, fcwA, wapP, wawP, wmwP = payload[p]
        in_maps.append({
            "x0t": np.ascontiguousarray(x0[b].T), "adw": adw, "qkw": qkwA,
            "qpw": qpwA, "vww": vwwA, "fcw": fcwA, "c16": c16, "cstf": cstf,
            "wapP": wapP, "wawP": wawP, "wmwP": wmwP, "rwP": rwP,
            "rbias2": rbias2, "lmt": lmt,
        })
    return active_sets, in_maps


def kernel(idx, n_steps, wte, adapters, qkv_w, attn_proj, mlp_fc, mlp_proj,
           dep, router_w, router_b, lm_head_w):
    active_sets, in_maps = _host_prep(
        idx, n_steps, wte, adapters, qkv_w, attn_proj, mlp_fc, mlp_proj,
        dep, router_w, router_b, lm_head_w)

    if active_sets not in _PROGRAM_CACHE:
        _PROGRAM_CACHE[active_sets] = _build_program(active_sets)
    nc = _PROGRAM_CACHE[active_sets]

    trace = bool(int(os.environ.get("BASS_KERNEL_TRACE", "0")))
    res = run_bass_kernel_spmd(nc, in_maps, list(range(NC)), trace=trace)
    if trace and res.exec_time_ns is not None:
        print(f"HW exec time: {res.exec_time_ns} ns")

    out = np.zeros((B, T, V), np.float32)
    for c in range(NC):
        b, p = c // VSH, c % VSH
        lo = p * VW
        hi = min(lo + VW, V)
        out[b, :, lo:hi] = 15.0 * res.results[c]["out_lg"][:, 0:hi - lo].astype(np.float32)
    return out
